# revision 2
# baseline (speedup 1.0000x reference)
"""Cross-covariance (XCA / channel) attention kernel for Trainium2, 8 NeuronCores.

Reference computation (per batch b, head h, with X = x[b] in R^{N x C}):
    qkv = X @ Wqkv + bqkv;  q,k,v per head as [hd, N] (channels x tokens)
    q <- l2norm(q, axis=N) * temp_h ; k <- l2norm(k, axis=N)
    attn = softmax(q @ k^T)                # [hd, hd] channel attention
    out_h = attn @ v                       # [hd, N]
    y = concat_h(out_h)^T @ Wproj + bproj  # [N, C]

Restructure (mathematically exact): all attention statistics derive from
the per-batch Gram matrix S = X^T X in R^{C x C}:
    G[h] = Wq_h^T S Wk_h,  ||q_d||^2 = diag(Wq_h^T S Wq_h), same for k
    attn[h] = softmax(temp_h * G[h] / (||q|| ||k||^T))
    y = X @ M + c,  M = sum_h Wv_h @ attn[h]^T @ Wproj_h

Sharding: 8 cores = 4 batches x 2 sequence halves. Every core computes the
FULL batch Gram matrix S locally (streams all 8192 tokens once) so no
cross-core collective is needed at all; the attention/M build (tiny) is
redundant within a pair, and each core produces its own 4096 output rows.

Phase 1 exploits S = S^T: only the upper-triangle chunk blocks are
accumulated (6 staircase PSUM tiles spanning exactly 8 banks, one pass
over x); the 15 lower 128x128 blocks come from PE transposes.

Matmuls run in float32r (fp32, 11-bit mantissa) at full PE rate. DMA is
spread over both HWDGE queues (sync+scalar) plus gpsimd for stores.
"""
import numpy as np

import concourse.bacc as bacc
import concourse.mybir as mybir
import concourse.tile as tile

B, N, C = 4, 8192, 768
H, HD = 12, 64
NLOC = N // 2          # tokens per core (4096)
NCORES = 8
F32 = mybir.dt.float32
F32R = mybir.dt.float32r
AX = mybir.AxisListType.X

_CACHE = {}


def _round_fp32r(a: np.ndarray) -> np.ndarray:
    """Round fp32 to fp32r (11-bit mantissa), round-half-to-even."""
    a = np.ascontiguousarray(a, dtype=np.float32)
    v = a.view(np.uint32)
    r = (v + np.uint32(0x7FF) + ((v >> np.uint32(12)) & np.uint32(1))) & np.uint32(0xFFFFF000)
    return r.view(np.float32)


def _build(has_bias: bool):
    nc = bacc.Bacc("TRN2", target_bir_lowering=False, debug=False,
                   enable_asserts=False, num_devices=NCORES)

    # ---- per-core I/O ----
    xf_d = nc.dram_tensor("xf", [N, C], F32R, kind="ExternalInput")      # full batch
    xT_d = nc.dram_tensor("xT", [C, NLOC], F32R, kind="ExternalInput")   # my half, T
    wqk_d = nc.dram_tensor("wqk", [C, 2 * C], F32R, kind="ExternalInput")
    wvt_d = nc.dram_tensor("wvt", [C, C], F32R, kind="ExternalInput")
    wproj_d = nc.dram_tensor("wproj", [C, C], F32R, kind="ExternalInput")
    tempv_d = nc.dram_tensor("tempv", [1, C], F32, kind="ExternalInput")
    ones128_d = nc.dram_tensor("ones128", [128, 1], F32R, kind="ExternalInput")
    ones1_d = nc.dram_tensor("ones1", [1, HD], F32R, kind="ExternalInput")
    ident_d = nc.dram_tensor("ident", [128, 128], F32R, kind="ExternalInput")
    if has_bias:
        gcorr_d = nc.dram_tensor("gcorr", [HD, C], F32, kind="ExternalInput")
        nq2c_d = nc.dram_tensor("nq2c", [1, C], F32, kind="ExternalInput")
        nk2c_d = nc.dram_tensor("nk2c", [1, C], F32, kind="ExternalInput")
        bvt_d = nc.dram_tensor("bvt", [128, H // 2], F32R, kind="ExternalInput")
        bproj_d = nc.dram_tensor("bproj", [1, C], F32, kind="ExternalInput")
        ones128w_d = nc.dram_tensor("ones128w", [1, 128], F32R, kind="ExternalInput")
    y_d = nc.dram_tensor("y", [NLOC, C], F32, kind="ExternalOutput")

    NT = N // 128        # 64 token chunks over the full batch
    KC = C // 128        # 6 channel chunks
    FH = C // 2          # 384
    TS = 256             # xT stream chunk (tokens)
    NB = NLOC // TS      # 16 blocks
    YB = 4               # y tiles per output DMA

    with tile.TileContext(nc) as tc:
        with (
            tc.tile_pool(name="const", bufs=1) as const,
            tc.tile_pool(name="small", bufs=1) as small,
            tc.tile_pool(name="sm2", bufs=2) as sm2,
            tc.tile_pool(name="wv", bufs=1) as wvp,
            tc.tile_pool(name="rm", bufs=1) as rmp,
            tc.tile_pool(name="xts", bufs=2) as xts,
            tc.tile_pool(name="dram", bufs=1, space="DRAM") as dram,
        ):
            # consts early on the scalar queue (tiny; ident needed mid-phase-1)
            ident_sb = const.tile([128, 128], F32R, tag="ident")
            nc.scalar.dma_start(out=ident_sb[:, :], in_=ident_d[:, :])
            ones128_sb = const.tile([128, 1], F32R, tag="ones128")
            nc.scalar.dma_start(out=ones128_sb[:, :], in_=ones128_d[:, :])
            ones1_sb = const.tile([1, HD], F32R, tag="ones1")
            nc.scalar.dma_start(out=ones1_sb[:, :], in_=ones1_d[:, :])
            tempv_sb = const.tile([1, C], F32, tag="tempv")
            nc.scalar.dma_start(out=tempv_sb[:, :], in_=tempv_d[:, :])

            g_sb = small.tile([HD, C], F32, tag="g")
            xT_view = xT_d.rearrange("(k p) n -> p k n", p=128)
            xt_tiles = []

            with tc.tile_pool(name="sr", bufs=1) as srp, \
                 tc.tile_pool(name="wqk", bufs=1) as wqkp:
                sr_sb = srp.tile([128, KC, C], F32R, tag="sr")
                wqk_sb = wqkp.tile([128, KC, 2 * C], F32R, tag="wqk")
                wqk_view = wqk_d.rearrange("(k p) c -> p k c", p=128)

                # ---------- phase 1: S = x^T x (upper triangle) ----------
                # staircase PSUM tiles T_i = S[128i.., 128i..C); per-partition
                # 3K+2.5K+2K+1.5K+1K+0.5K bytes -> exactly 8 banks.
                x_view = xf_d.rearrange("(t p) c -> p t c", p=128)
                with tc.tile_pool(name="xs", bufs=6) as xs, \
                     tc.tile_pool(name="ps1", bufs=1, space="PSUM") as ps1:
                    t_ps = [ps1.tile([128, C - 128 * i], F32, tag=f"t{i}",
                                     name=f"t_ps{i}")
                            for i in range(KC)]
                    for t in range(NT):
                        x_t = xs.tile([128, C], F32R, tag="xt")
                        eng = nc.sync if t % 2 == 0 else nc.scalar
                        eng.dma_start(out=x_t[:, :], in_=x_view[:, t, :])
                        # interleave weight chunks into the DMA slack
                        if t in (16, 20, 24, 28, 32, 36):
                            k = (t - 16) // 4
                            nc.sync.dma_start(out=wqk_sb[:, k, :],
                                              in_=wqk_view[:, k, :])
                        st, sp = (t == 0), (t == NT - 1)
                        for i in range(KC):
                            w = C - 128 * i
                            # sub-ranges bank-aligned (512 f32 = one 2KB bank)
                            for c0 in range(0, w, 512):
                                c1 = min(c0 + 512, w)
                                nc.tensor.matmul(
                                    t_ps[i][:, c0:c1],
                                    x_t[:, 128 * i:128 * (i + 1)],
                                    x_t[:, 128 * i + c0:128 * i + c1],
                                    start=st, stop=sp)
                    # upper blocks PSUM -> SBUF (f32r), alternate engines
                    for i in range(KC):
                        if i % 2 == 0:
                            nc.vector.tensor_copy(sr_sb[:, i, 128 * i:C],
                                                  t_ps[i][:, :])
                        else:
                            nc.scalar.copy(sr_sb[:, i, 128 * i:C], t_ps[i][:, :])

                # remaining big loads, queued now (scalar queue is free)
                wvt_sb = wvp.tile([128, KC, C], F32R, tag="wvt")
                nc.scalar.dma_start(out=wvt_sb[:, :, :],
                                    in_=wvt_d.rearrange("(k p) c -> p k c", p=128))
                wproj_sb = wvp.tile([128, KC, C], F32R, tag="wproj")
                nc.scalar.dma_start(out=wproj_sb[:, :, :],
                                    in_=wproj_d.rearrange("(k p) c -> p k c", p=128))
                # prefetch first xT blocks for phase 4 (sync queue)
                for bb in range(2):
                    xt_t = xts.tile([128, KC, TS], F32R, tag="xTs", name=f"xt{bb}")
                    nc.sync.dma_start(out=xt_t[:, :, :],
                                      in_=xT_view[:, :, bb * TS:(bb + 1) * TS])
                    xt_tiles.append(xt_t)

                # lower blocks of S by PE transpose of the upper ones
                with tc.tile_pool(name="pst", bufs=4, space="PSUM") as pst:
                    for i in range(KC):
                        for j in range(i + 1, KC):
                            tp = pst.tile([128, 128], F32R, tag="tp")
                            nc.tensor.transpose(tp[:, :],
                                                sr_sb[:, i, 128 * j:128 * (j + 1)],
                                                ident_sb[:, :])
                            if (i + j) % 2 == 0:
                                nc.vector.tensor_copy(
                                    sr_sb[:, j, 128 * i:128 * (i + 1)], tp[:, :])
                            else:
                                nc.scalar.copy(
                                    sr_sb[:, j, 128 * i:128 * (i + 1)], tp[:, :])

                # ---------- phase 2: U = S @ [Wq|Wk], norms, G ----------
                n2_sb = small.tile([1, 2 * C], F32, tag="n2")
                with tc.tile_pool(name="uk", bufs=1) as ukp, \
                     tc.tile_pool(name="pp", bufs=2) as pp:
                    uk_sb = ukp.tile([128, KC, C], F32R, tag="uk")
                    with tc.tile_pool(name="ps2", bufs=1, space="PSUM") as ps2:
                        u_tags = [ps2.tile([128, C], F32, tag=f"u{i}",
                                           name=f"u_ps{i}") for i in range(2)]
                        n_tags = [ps2.tile([1, C], F32, tag=f"n{i}",
                                           name=f"n_ps{i}") for i in range(2)]
                        for half in range(2):        # 0: q-norms, 1: k-norms+Uk
                            co = half * C
                            for m in range(KC):
                                u_ps = u_tags[m % 2]
                                for k in range(KC):
                                    for c0 in range(0, C, 512):
                                        c1 = min(c0 + 512, C)
                                        nc.tensor.matmul(
                                            u_ps[:, c0:c1],
                                            sr_sb[:, k, 128 * m:128 * (m + 1)],
                                            wqk_sb[:, k, co + c0:co + c1],
                                            start=(k == 0), stop=(k == KC - 1))
                                p_sb = pp.tile([128, C], F32R, tag="p")
                                nc.vector.tensor_mul(
                                    p_sb[:, :],
                                    wqk_sb.bitcast(F32)[:, m, co:co + C],
                                    u_ps[:, :])
                                for c0 in range(0, C, 512):
                                    c1 = min(c0 + 512, C)
                                    nc.tensor.matmul(
                                        n_tags[half][:, c0:c1],
                                        ones128_sb[:, :], p_sb[:, c0:c1],
                                        start=(m == 0), stop=(m == KC - 1))
                                if half == 1:
                                    nc.scalar.copy(uk_sb[:, m, :], u_ps[:, :])
                        for half in range(2):
                            nc.vector.tensor_copy(
                                n2_sb[:, half * C:(half + 1) * C],
                                n_tags[half][:, :])

                    # G[h] = Wq_h^T Uk_h -> [64, (h,e)]
                    with tc.tile_pool(name="ps3", bufs=4, space="PSUM") as ps3:
                        for h in range(H):
                            g_ps = ps3.tile([HD, HD], F32, tag="g")
                            for k in range(KC):
                                nc.tensor.matmul(
                                    g_ps[:, :],
                                    wqk_sb[:, k, h * HD:(h + 1) * HD],
                                    uk_sb[:, k, h * HD:(h + 1) * HD],
                                    start=(k == 0), stop=(k == KC - 1))
                            nc.vector.tensor_copy(g_sb[:, h * HD:(h + 1) * HD],
                                                  g_ps[:, :])
            # sr/wqk/uk released here

            if has_bias:
                nq2c_sb = const.tile([1, C], F32, tag="nq2c")
                nc.scalar.dma_start(out=nq2c_sb[:, :], in_=nq2c_d[:, :])
                nk2c_sb = const.tile([1, C], F32, tag="nk2c")
                nc.scalar.dma_start(out=nk2c_sb[:, :], in_=nk2c_d[:, :])
                gcorr_sb = const.tile([HD, C], F32, tag="gcorr")
                nc.scalar.dma_start(out=gcorr_sb[:, :], in_=gcorr_d[:, :])
                nc.vector.tensor_add(n2_sb[:, 0:C], n2_sb[:, 0:C], nq2c_sb[:, :])
                nc.vector.tensor_add(n2_sb[:, C:], n2_sb[:, C:], nk2c_sb[:, :])
                nc.vector.tensor_add(g_sb[:, :], g_sb[:, :], gcorr_sb[:, :])

            # ---------- phase 3: softmax + R + M ----------
            ns_sb = small.tile([1, 2 * C], F32, tag="ns")
            nc.scalar.activation(ns_sb[:, :], n2_sb[:, :],
                                 mybir.ActivationFunctionType.Sqrt)
            tnq_sb = sm2.tile([1, C], F32, tag="onec", name="tnq")
            nc.vector.reciprocal(tnq_sb[:, :], ns_sb[:, 0:C])
            tnqf_sb = sm2.tile([1, C], F32, tag="onec", name="tnqf")
            nc.vector.tensor_mul(tnqf_sb[:, :], tnq_sb[:, :], tempv_sb[:, :])
            # rearrange [1,(h,d)] -> [d, h] via DRAM round-trip (gpsimd queue)
            scr = dram.tile([1, C], F32)
            nc.gpsimd.dma_start(out=scr[:, :], in_=tnqf_sb[:, :])
            tnqT_sb = small.tile([HD, H], F32, tag="tnqT")
            nc.gpsimd.dma_start(out=tnqT_sb[:, :],
                                in_=scr.rearrange("one (h d) -> (one d) h", d=HD))
            nkinv_sb = sm2.tile([1, C], F32R, tag="onec", name="nkinv")
            with nc.allow_low_precision(reason="fp32r rounding of 1/||k|| ok"):
                nc.vector.reciprocal(nkinv_sb[:, :], ns_sb[:, C:])

            with tc.tile_pool(name="ps3b", bufs=1, space="PSUM") as ps3b:
                nkbc_ps = []
                for f in range(2):
                    b_ps = ps3b.tile([HD, FH], F32, tag=f"nkbc{f}")
                    nc.tensor.matmul(b_ps[:, :], ones1_sb[:, :],
                                     nkinv_sb[:, f * FH:(f + 1) * FH],
                                     start=True, stop=True)
                    nkbc_ps.append(b_ps)
                t1_sb = sm2.tile([HD, C], F32, tag="hdc", name="t1")
                for f in range(2):
                    nc.vector.tensor_mul(t1_sb[:, f * FH:(f + 1) * FH],
                                         g_sb[:, f * FH:(f + 1) * FH],
                                         nkbc_ps[f][:, :])
            t2_sb = sm2.tile([HD, H, HD], F32, tag="hdc", name="t2")
            nc.vector.tensor_mul(
                t2_sb[:, :, :],
                t1_sb.rearrange("d (h e) -> d h e", h=H),
                tnqT_sb.unsqueeze(2).broadcast_to([HD, H, HD]))
            # |logits| <= max|temp| (Cauchy-Schwarz on normalized vectors):
            # safe to exp without max-subtraction for the given inputs.
            e_sb = sm2.tile([HD, H, HD], F32, tag="hdc", name="e")
            nc.scalar.activation(e_sb[:, :, :], t2_sb[:, :, :],
                                 mybir.ActivationFunctionType.Exp)
            sum_sb = small.tile([HD, H], F32, tag="sum")
            nc.vector.reduce_sum(sum_sb[:, :], e_sb[:, :, :], AX)
            rec_sb = small.tile([HD, H], F32, tag="rec")
            nc.vector.reciprocal(rec_sb[:, :], sum_sb[:, :])
            attn_sb = sm2.tile([HD, H, HD], F32R, tag="hdc", name="attn")
            nc.vector.tensor_mul(
                attn_sb[:, :, :], e_sb[:, :, :],
                rec_sb.unsqueeze(2).broadcast_to([HD, H, HD]))
            # parity-split attn so lhsT base partition matches Wproj rows:
            # attn2[(h%2)*64+d, h//2, e] = attn[d, h, e]
            attn2_sb = small.tile([128, H // 2, HD], F32R, tag="attn2")
            av = attn_sb.rearrange("d (j two) e -> d two j e", two=2)
            for p0 in range(2):
                nc.gpsimd.dma_start(out=attn2_sb[p0 * HD:(p0 + 1) * HD, :, :],
                                    in_=av[:, p0, :, :])

            # R_h = attn_h^T @ Wproj_h   [64(e), C]
            r_sb = rmp.tile([128, KC, C], F32R, tag="r")
            with tc.tile_pool(name="ps3c", bufs=4, space="PSUM") as ps3c:
                for h in range(H):
                    po, pc = (h % 2) * HD, h // 2
                    for f in range(2):
                        r_ps = ps3c.tile([HD, FH], F32, tag="r")
                        nc.tensor.matmul(r_ps[:, :],
                                         attn2_sb[po:po + HD, pc, :],
                                         wproj_sb[po:po + HD, pc,
                                                  f * FH:(f + 1) * FH],
                                         start=True, stop=True)
                        nc.vector.tensor_copy(
                            r_sb[po:po + HD, pc, f * FH:(f + 1) * FH], r_ps[:, :])

            # M = sum_h Wv_h @ R_h = wvt^T @ R  (full 128-part k-chunks)
            m_sb = rmp.tile([128, KC, C], F32R, tag="m")
            crow_sb = None
            ones128w_sb = None
            with tc.tile_pool(name="ps4", bufs=2, space="PSUM") as ps4:
                for m in range(KC):
                    m_ps = ps4.tile([128, C], F32, tag="m")
                    for k in range(KC):
                        for c0 in range(0, C, 512):
                            c1 = min(c0 + 512, C)
                            nc.tensor.matmul(m_ps[:, c0:c1],
                                             wvt_sb[:, k, 128 * m:128 * (m + 1)],
                                             r_sb[:, k, c0:c1],
                                             start=(k == 0), stop=(k == KC - 1))
                    nc.vector.tensor_copy(m_sb[:, m, :], m_ps[:, :])

                if has_bias:
                    # c = sum_h bv_h^T R_h + bproj  (bvt rows (h%2)*64+e)
                    bvt_sb = const.tile([128, H // 2], F32R, tag="bvt")
                    nc.scalar.dma_start(out=bvt_sb[:, :], in_=bvt_d[:, :])
                    bproj_sb = const.tile([1, C], F32, tag="bproj")
                    nc.scalar.dma_start(out=bproj_sb[:, :], in_=bproj_d[:, :])
                    ones128w_sb = const.tile([1, 128], F32R, tag="ones128w")
                    nc.scalar.dma_start(out=ones128w_sb[:, :], in_=ones128w_d[:, :])
                    crow_sb = sm2.tile([1, C], F32R, tag="onec", name="crow")
                    for f in range(2):
                        c_ps0 = ps4.tile([1, FH], F32, tag="c0", name=f"c0_{f}")
                        c_ps1 = ps4.tile([1, FH], F32, tag="c1", name=f"c1_{f}")
                        for j in range(H // 2):
                            nc.tensor.matmul(c_ps0[:, :], bvt_sb[0:HD, j:j + 1],
                                             r_sb[0:HD, j, f * FH:(f + 1) * FH],
                                             start=(j == 0), stop=(j == H // 2 - 1))
                        for j in range(H // 2):
                            nc.tensor.matmul(c_ps1[:, :], bvt_sb[HD:128, j:j + 1],
                                             r_sb[HD:128, j, f * FH:(f + 1) * FH],
                                             start=(j == 0), stop=(j == H // 2 - 1))
                        tmpc_sb = small.tile([1, FH], F32, tag="tmpc", name=f"tc{f}")
                        nc.vector.tensor_copy(tmpc_sb[:, :], c_ps0[:, :])
                        nc.vector.tensor_tensor(tmpc_sb[:, :], tmpc_sb[:, :],
                                                c_ps1[:, :], mybir.AluOpType.add)
                        nc.vector.tensor_tensor(crow_sb[:, f * FH:(f + 1) * FH],
                                                tmpc_sb[:, :],
                                                bproj_sb[:, f * FH:(f + 1) * FH],
                                                mybir.AluOpType.add)

            # ---------- phase 4: y = x @ M (+ c) ----------
            yv = y_d.rearrange("(t p) c -> p t c", p=128)
            with tc.tile_pool(name="yo", bufs=2) as yo, \
                 tc.tile_pool(name="ps5", bufs=3, space="PSUM") as ps5:
                y_sb = None
                for tt in range(NLOC // 128):
                    bb, sub = tt // (TS // 128), tt % (TS // 128)
                    if sub == 0 and bb + 2 < NB:
                        xt_t = xts.tile([128, KC, TS], F32R, tag="xTs",
                                        name=f"xt{bb + 2}")
                        nc.sync.dma_start(
                            out=xt_t[:, :, :],
                            in_=xT_view[:, :, (bb + 2) * TS:(bb + 3) * TS])
                        xt_tiles.append(xt_t)
                    xt_t = xt_tiles[bb]
                    if tt % YB == 0:
                        y_sb = yo.tile([128, YB, C], F32, tag="y")
                    y_ps = ps5.tile([128, C], F32, tag="y")
                    for k in range(KC):
                        for c0 in range(0, C, 512):
                            c1 = min(c0 + 512, C)
                            nc.tensor.matmul(y_ps[:, c0:c1],
                                             xt_t[:, k, sub * 128:(sub + 1) * 128],
                                             m_sb[:, k, c0:c1],
                                             start=(k == 0),
                                             stop=(k == KC - 1 and not has_bias))
                    if has_bias:
                        for c0 in range(0, C, 512):
                            c1 = min(c0 + 512, C)
                            nc.tensor.matmul(y_ps[:, c0:c1], ones128w_sb[:, :],
                                             crow_sb[:, c0:c1],
                                             start=False, stop=True)
                    if tt % 2 == 0:
                        nc.vector.tensor_copy(y_sb[:, tt % YB, :], y_ps[:, :])
                    else:
                        nc.scalar.copy(y_sb[:, tt % YB, :], y_ps[:, :])
                    if tt % YB == YB - 1:
                        g0 = tt - (YB - 1)
                        nc.gpsimd.dma_start(out=yv[:, g0:g0 + YB, :],
                                            in_=y_sb[:, :, :])

    nc.compile()
    return nc


def _get_program(has_bias: bool):
    if has_bias not in _CACHE:
        _CACHE[has_bias] = _build(has_bias)
    return _CACHE[has_bias]


def _prepare_inputs(x, Wqkv, bqkv, temperature, Wproj, bproj, has_bias):
    """Build the 8 per-core input maps (host-side sharding + fp32r prep)."""
    x = np.asarray(x, np.float32)
    Wqkv = np.asarray(Wqkv, np.float32)
    bqkv = np.asarray(bqkv, np.float32)
    temperature = np.asarray(temperature, np.float32)
    Wproj = np.asarray(Wproj, np.float32)
    bproj = np.asarray(bproj, np.float32)

    wqk = _round_fp32r(Wqkv[:, :2 * C])
    wvt = _round_fp32r(np.ascontiguousarray(Wqkv[:, 2 * C:].T))
    wproj = _round_fp32r(Wproj)
    tempv = np.repeat(temperature.reshape(H), HD).reshape(1, C).astype(np.float32)

    common = dict(wqk=wqk, wvt=wvt, wproj=wproj, tempv=tempv,
                  ones128=np.ones((128, 1), np.float32),
                  ones1=np.ones((1, HD), np.float32),
                  ident=np.eye(128, dtype=np.float32))

    if has_bias:
        bq, bk, bv = bqkv[:C], bqkv[C:2 * C], bqkv[2 * C:]
        colsum = x.sum(axis=1, dtype=np.float64)            # [B, C]
        common["bvt"] = _round_fp32r(bv.reshape(H, HD).T.copy())
        common["bproj"] = bproj.reshape(1, C)
        common["ones128w"] = np.ones((1, 128), np.float32)

    xr = [_round_fp32r(x[b]) for b in range(B)]
    in_maps = []
    for core in range(NCORES):
        b, half = core // 2, core % 2
        m = dict(common)
        m["xf"] = xr[b]
        m["xT"] = np.ascontiguousarray(xr[b][half * NLOC:(half + 1) * NLOC, :].T)
        if has_bias:
            cs = colsum[b]                                   # [C]
            gc = np.zeros((HD, C), np.float32)
            nq2c = np.zeros((1, C), np.float32)
            nk2c = np.zeros((1, C), np.float32)
            for h in range(H):
                sl = slice(h * HD, (h + 1) * HD)
                csWk = cs @ Wqkv[:, C + h * HD:C + (h + 1) * HD].astype(np.float64)
                csWq = cs @ Wqkv[:, h * HD:(h + 1) * HD].astype(np.float64)
                gc[:, sl] = (np.outer(bq[sl], csWk) + np.outer(csWq, bk[sl])
                             + N * np.outer(bq[sl], bk[sl])).astype(np.float32)
                nq2c[0, sl] = (2 * bq[sl] * csWq + N * bq[sl] ** 2).astype(np.float32)
                nk2c[0, sl] = (2 * bk[sl] * csWk + N * bk[sl] ** 2).astype(np.float32)
            m["gcorr"] = gc
            m["nq2c"] = nq2c
            m["nk2c"] = nk2c
        in_maps.append(m)
    return in_maps


def kernel(x, Wqkv, bqkv, temperature, Wproj, bproj):
    from concourse import bass2jax
    has_bias = bool(np.any(np.asarray(bqkv)) or np.any(np.asarray(bproj)))
    nc = _get_program(has_bias)
    in_maps = _prepare_inputs(x, Wqkv, bqkv, temperature, Wproj, bproj, has_bias)
    results = bass2jax.run_bass_via_pjrt(nc, in_maps, n_cores=NCORES)
    out = np.empty((B, N, C), np.float32)
    for core in range(NCORES):
        b, half = core // 2, core % 2
        out[b, half * NLOC:(half + 1) * NLOC, :] = results[core]["y"]
    return out
